# revision 8
# baseline (speedup 1.0000x reference)
"""Multi-head attention kernel for Trainium2, 8-core SPMD.

Problem: q,k,v [B=2, H=16, S=2048, D=128] fp32 ->
         softmax(q@k^T/sqrt(D)) @ v, same shape.

Sharding: 32 (b,h) pairs split across 8 cores -> 4 heads per core, each
core computing full attention for its heads independently (no comms).

Per-core pipeline (per head):
  - load Q,K,V fp32 (sync/HWDGE), cast to bf16 on DVE
  - xbar-transpose Q,K -> [d=128, s=2048]
  - per q-tile (128 rows): S = Q K^T via 4 matmuls N=512 into PSUM,
    ACT exp(scale*S) PSUM->SBUF bf16 with accum_out row-sums,
    xbar-transpose P tile -> P^T blocks [k=128, 16, q]
  - per q-chunk (512 rows): O^T = sum_j V_j^T P^T_j (16 matmuls N=512),
    evac bf16, xbar-transpose to O [q, d], scale by 1/rowsum on DVE,
    DMA out fp32.

All DMA (copies and xbar transposes) is issued on the sync (SP) engine so
they share one HWDGE FIFO: the DmaCopy<->DmaTranspose xbar-mode hazard is
then ordered by the queue itself and Tile emits no cross-queue semaphore
waits for it. This matters because the DMA_DIRECT2D_XPOSE instruction only
supports a single sync-wait slot in hardware; each transpose must end up
with at most one semaphore wait. For the P^T target buffer (reused every
other chunk) the write-after-read release comes from the PE (the O^T
matmuls), which would be a second wait on the transpose - an SP nop
"absorber" takes that wait instead, ordered before the transposes.
"""

import numpy as np

import concourse.bass as bass
import concourse.mybir as mybir
import concourse.tile as tile
from concourse.tile import add_dep_helper

NCORES = 8
B, H, S, D = 2, 16, 2048, 128
HPC = (B * H) // NCORES  # heads per core = 4
P = 128                  # partitions / tile rows
NT = S // P              # 16 q/k tiles per head
NG = S // 512            # 4 q-chunks of 512
SCALE = 1.0 / float(np.sqrt(D))

F32 = mybir.dt.float32
BF16 = mybir.dt.bfloat16
EXP = mybir.ActivationFunctionType.Exp


def _absorbed_xbar(nc, dst, src, deps):
    """DMA-xbar transpose with <=1 HW sync wait.

    DMA_DIRECT2D_XPOSE has a single sync-wait slot. Tile's wait emission is
    not transitively minimal, so a transpose whose ancestors span several
    processors (producer engine + DMA-completion lanes + WAR releasers)
    would get several waits. An SP nop that depends on all of `deps` is
    ordered before the transpose on the same queue; it takes the waits and
    advances SP's observed clock, so the transpose itself needs none.
    """
    nop = nc.sync.nop()
    for d in deps:
        if d is not None:
            add_dep_helper(nop.ins, d.ins, reason="xpose wait absorber")
    t = nc.sync.dma_start(dst, src, transpose=True)
    add_dep_helper(t.ins, nop.ins, reason="xpose after absorber")
    return t


def attention_tiles(tc: "tile.TileContext", q, k, v, o):
    nc = tc.nc
    with (
        tc.tile_pool(name="stage", bufs=3) as stp,      # fp32 load staging
        tc.tile_pool(name="natb", bufs=4) as natp,      # bf16 q/k natural
        tc.tile_pool(name="vn", bufs=4) as vnp,
        tc.tile_pool(name="qt", bufs=4) as qtp,
        tc.tile_pool(name="kt", bufs=4) as ktp,
        tc.tile_pool(name="spsum", bufs=3, space="PSUM") as spp,
        tc.tile_pool(name="otpsum", bufs=2, space="PSUM") as otp,
        tc.tile_pool(name="pb", bufs=3) as pbp,
        tc.tile_pool(name="ptg", bufs=2) as ptp,
        tc.tile_pool(name="otsb", bufs=2) as otsbp,
        tc.tile_pool(name="otr", bufs=2) as otrp,
        tc.tile_pool(name="osb", bufs=2) as osbp,
        tc.tile_pool(name="racc", bufs=4) as raccp,
        tc.tile_pool(name="rr", bufs=8) as rrp,
    ):
        last_ot_mm = {}   # global chunk index -> last O^T matmul instruction
        last_osb_mul = {}  # global chunk index -> osb scale (otr WAR releaser)
        prev_pxbar = None  # anti-hoist anchor for the next head's loads

        for h in range(HPC):
            # --- load fp32 (SP queue), cast to bf16 on DVE ---
            qf = stp.tile([P, NT, D], F32, tag="stage")
            kf = stp.tile([P, NT, D], F32, tag="stage")
            vf = stp.tile([P, NT, D], F32, tag="stage")
            ld_q = nc.sync.dma_start(qf[:], q[h].rearrange("(t p) d -> p t d", p=P))
            ld_k = nc.sync.dma_start(kf[:], k[h].rearrange("(t p) d -> p t d", p=P))
            ld_v = nc.sync.dma_start(vf[:], v[h].rearrange("(t p) d -> p t d", p=P))
            if prev_pxbar is not None:
                # Keep loads from being scheduled between earlier transposes:
                # an SBUF-writing copy amid xposes re-adds hazard waits there.
                for ld in (ld_q, ld_k, ld_v):
                    add_dep_helper(ld.ins, prev_pxbar.ins, reason="load anti-hoist")
            qb = natp.tile([P, NT, D], BF16, tag="natb")
            kb = natp.tile([P, NT, D], BF16, tag="natb")
            vn = vnp.tile([P, NT, D], BF16)
            c_q = nc.vector.tensor_copy(qb[:], qf[:])
            c_k = nc.vector.tensor_copy(kb[:], kf[:])
            nc.vector.tensor_copy(vn[:], vf[:])

            # --- transpose Q,K -> [d, s] ---
            qt = qtp.tile([P, NT, P], BF16)   # qt[d, t, qq] = Q[t*128+qq, d]
            kt = ktp.tile([P, NT, P], BF16)   # kt[d, t, kk] = K[t*128+kk, d]
            _absorbed_xbar(nc, qt[:], qb[:], [c_q, c_k, ld_q, ld_k, ld_v])
            _absorbed_xbar(nc, kt[:], kb[:], [c_k])

            for g in range(NG):  # q-chunks of 512
                ci = h * NG + g  # global chunk index

                # P^T chunk: ptg[kk, j, li*128+qq] = P[li*128+qq, j*128+kk]
                ptg = ptp.tile([P, NT, 512], BF16)
                racc = raccp.tile([P, 8], F32)  # exp row sums, col = half*4+li

                for li in range(4):
                    qi = g * 4 + li
                    pb = pbp.tile([P, S], BF16)
                    exps = []
                    for half in range(2):
                        sp = spp.tile([P, 1024], F32)
                        for jj in range(2):
                            c = half * 2 + jj
                            nc.tensor.matmul(
                                sp[:, jj * 512:(jj + 1) * 512],
                                lhsT=qt[:, qi, :],
                                rhs=kt[:, c * 4:(c + 1) * 4, :],
                                start=True,
                                stop=True,
                            )
                        exps.append(
                            nc.scalar.activation(
                                pb[:, half * 1024:(half + 1) * 1024],
                                sp[:],
                                EXP,
                                scale=SCALE,
                                accum_out=racc[:, half * 4 + li:half * 4 + li + 1],
                            )
                        )
                    deps = list(exps)
                    if li == 0 and ci - 2 in last_ot_mm:
                        deps.append(last_ot_mm[ci - 2])  # ptg slot WAR release
                    prev_pxbar = _absorbed_xbar(
                        nc, ptg[:, :, li * P:(li + 1) * P], pb[:], deps
                    )

                # row-sum reciprocal for the chunk
                rsum = rrp.tile([P, 4], F32, tag="rsum")
                rrec = rrp.tile([P, 4], F32, tag="rrec")
                nc.vector.tensor_add(rsum[:], racc[:, 0:4], racc[:, 4:8])
                nc.vector.reciprocal(rrec[:], rsum[:])

                # O^T accumulation over the 16 k-tiles
                ot = otp.tile([P, 512], F32)
                for j in range(NT):
                    mm = nc.tensor.matmul(
                        ot[:],
                        lhsT=vn[:, j, :],
                        rhs=ptg[:, j, :],
                        start=(j == 0),
                        stop=(j == NT - 1),
                    )
                last_ot_mm[ci] = mm

                otsb = otsbp.tile([P, 512], BF16)
                cp_ot = nc.vector.tensor_copy(otsb[:], ot[:])
                otr = otrp.tile([P, 4, P], BF16)  # otr[qq, li, d] = O[...]
                _absorbed_xbar(
                    nc, otr[:], otsb[:], [cp_ot, last_osb_mul.get(ci - 2)]
                )

                osb = osbp.tile([P, 4, P], F32)
                last_osb_mul[ci] = nc.vector.tensor_mul(
                    osb[:], otr[:], rrec[:, :, None].to_broadcast([P, 4, P])
                )
                nc.sync.dma_start(
                    o[h].rearrange("(g t p) d -> g p t d", p=P, t=4)[g], osb[:]
                )


def build_nc():
    nc = bass.Bass()
    q = nc.declare_dram_parameter("q", [HPC, S, D], F32, isOutput=False)
    k = nc.declare_dram_parameter("k", [HPC, S, D], F32, isOutput=False)
    v = nc.declare_dram_parameter("v", [HPC, S, D], F32, isOutput=False)
    o = nc.declare_dram_parameter("o", [HPC, S, D], F32, isOutput=True)
    with tile.TileContext(nc) as tc:
        attention_tiles(tc, q.ap(), k.ap(), v.ap(), o.ap())
    # Legalize sync waits: DMA_DIRECT2D_XPOSE (and friends) only support a
    # single HW sync-wait slot; this splits multi-wait instructions into
    # EventSemaphore chains (same pass bacc runs for raw-bass kernels).
    import bass_rust

    bass_rust.generate_event_semaphores(nc)
    return nc


_NC_CACHE = None


def get_nc():
    global _NC_CACHE
    if _NC_CACHE is None:
        _NC_CACHE = build_nc()
    return _NC_CACHE


def shard_inputs(q, k, v):
    """Full [B,H,S,D] -> list of per-core input dicts."""
    qf = np.ascontiguousarray(np.asarray(q, dtype=np.float32).reshape(B * H, S, D))
    kf = np.ascontiguousarray(np.asarray(k, dtype=np.float32).reshape(B * H, S, D))
    vf = np.ascontiguousarray(np.asarray(v, dtype=np.float32).reshape(B * H, S, D))
    maps = []
    for c in range(NCORES):
        sl = slice(c * HPC, (c + 1) * HPC)
        maps.append(
            {
                "q": np.ascontiguousarray(qf[sl]),
                "k": np.ascontiguousarray(kf[sl]),
                "v": np.ascontiguousarray(vf[sl]),
            }
        )
    return maps


def unshard_output(results):
    """List of per-core {'o': [HPC,S,D]} -> full [B,H,S,D] fp32."""
    out = np.empty((B * H, S, D), dtype=np.float32)
    for c in range(NCORES):
        out[c * HPC:(c + 1) * HPC] = np.asarray(results[c]["o"], dtype=np.float32)
    return out.reshape(B, H, S, D)


def kernel(q, k, v):
    from concourse.bass_utils import run_bass_kernel_spmd

    nc = get_nc()
    in_maps = shard_inputs(q, k, v)
    res = run_bass_kernel_spmd(nc, in_maps, list(range(NCORES)))
    return unshard_output(res.results)


if __name__ == "__main__":
    rng = np.random.default_rng(0)
    q = rng.standard_normal((B, H, S, D), dtype=np.float32)
    k = rng.standard_normal((B, H, S, D), dtype=np.float32)
    v = rng.standard_normal((B, H, S, D), dtype=np.float32)
    out = kernel(q, k, v)
    print("out", out.shape, out.dtype, float(np.abs(out).max()))
